# revision 1
# baseline (speedup 1.0000x reference)
"""YOLO-style detection decode (nms_detection) on 8 trn2 NeuronCores.

Data-parallel over batch (64 -> 8 images/core). All per-core inputs are
packed into ONE flat f32 DRAM tensor (x52|x26|x13 in natural [b,ch,s]
order, then small constants) and the result is ONE [28392, 18] f32
tensor (cells x (3 anchors x 6)), reassembled on the host. One input +
one output minimizes the large per-tensor dispatch overhead of the
execution path.

Device pipeline per 4-chunk group (chunk = 128 cells):
  - segment DMAs load [128ch, cells] strips (raw channel order).
  - PE transposes chunks into PSUM -> [cell, 255ch].
  - DVE reduce_max over the 80 class cols per anchor -> m (exact).
  - PE transposes m; an fp32 K=3 matmul subtracts m from the class
    logits (exact: Sterbenz near the max) and a K=1 matmul adds
    (79-c)*2^-31. The winner's value is then exactly
    (79-argmax)*2^-31 >= 0 while every loser stays < 0, so a second
    DVE reduce_max recovers argmax exactly (incl. first-index ties,
    matching jnp.argmax).
  - decode: conf = sigmoid (ACT), cx/cy fused scalar_tensor_tensor with
    host grid offsets, w/h = exp * anchors/416, mask = (logit > 0)
    applied multiplicatively (fused is_gt*mult per anchor).
"""

import os
from contextlib import ExitStack

import numpy as np

import concourse.bass as bass
import concourse.tile as tile
from concourse import bacc, mybir
from concourse.bass_utils import run_bass_kernel_spmd

N_CORES = 8
B = 64
B_PER = B // N_CORES
CASE = 416.0
SCALES = [("52", 52, 8.0), ("26", 26, 16.0), ("13", 13, 32.0)]
CHUNK = 128
GRP = 4
F32 = mybir.dt.float32
AX = mybir.AxisListType
OP = mybir.AluOpType
AF = mybir.ActivationFunctionType
IOTA_SCALE = 2.0 ** -31


def _cells(h):
    return B_PER * h * h


def _nchunks(h):
    return (_cells(h) + CHUNK - 1) // CHUNK


def _gxy_section(h, t):
    n = _cells(h)
    nch = _nchunks(h)
    cells = np.arange(nch * CHUNK)
    s = cells % (h * h)
    gx = (s % h).astype(np.float64) * t / CASE
    gy = (s // h).astype(np.float64) * t / CASE
    gx[cells >= n] = 0.0
    gy[cells >= n] = 0.0
    out = np.zeros((CHUNK, 2 * nch), np.float32)
    for j in range(nch):
        out[:, 2 * j] = gx[j * CHUNK:(j + 1) * CHUNK]
        out[:, 2 * j + 1] = gy[j * CHUNK:(j + 1) * CHUNK]
    return out


def _consts():
    import ml_dtypes
    bf = ml_dtypes.bfloat16
    # raw channel order: anchor a's class cols at 85a+5 .. 85a+85.
    # sel9 rows 32q + (3*term + a): -1 selector for the 3-term bf16 split.
    sel9 = np.zeros((128, 256), bf)
    for q in range(4):
        for r in range(9):
            a = r % 3
            sel9[32 * q + r, 85 * a + 5:85 * a + 85] = -1.0
    iotam = np.zeros((1, 256), bf)
    for a in range(3):
        iotam[0, 85 * a + 5:85 * a + 85] = \
            ((79.0 - np.arange(80)) * IOTA_SCALE).astype(bf)
    onesb = np.ones((1, 128), bf)
    iden = np.eye(128, dtype=np.float32)
    gxy = np.concatenate([_gxy_section(h, t) for _, h, t in SCALES], axis=1)
    return {
        "gxy": gxy.astype(np.float32),
        "iden": iden,
        "sel9": sel9.view(np.float32),
        "iotam": iotam.view(np.float32),
        "onesb": onesb.view(np.float32),
    }


_CONSTS = _consts()

# packed input layout (f32 elements, per core)
_X_OFF = {}
_off = 0
for _tag, _h, _t in SCALES:
    _X_OFF[_tag] = _off
    _off += B_PER * 255 * _h * _h
_CONST_OFF = {}
for _name in ("gxy", "iden", "sel9", "iotam", "onesb"):
    _CONST_OFF[_name] = _off
    _off += _CONSTS[_name].size
_CONST_OFF["anch"] = _off
_off += 128 * 18
TOTAL_IN = _off

_O_OFF = {}
_off = 0
for _tag, _h, _t in SCALES:
    _O_OFF[_tag] = _off
    _off += _cells(_h)
TOTAL_OUT_ROWS = _off  # 28392


def _a85(ap_pgx, lo, width=1):
    """[128, gc, 3(anchor), width] view of box channel `lo` from a
    [128, gc, 512] psum group view (channel stride 85)."""
    v = ap_pgx[:, :, 0:255].rearrange("p g (a r) -> p g a r", a=3, r=85)
    return v[:, :, :, lo:lo + width]


def _emit_scale(nc, tc, ctx, pools, sb, xin, oX, h, t, tag, gxy_off):
    ST = int(os.environ.get("KSTAGE", "9"))
    n = _cells(h)
    hw = h * h
    nch = _nchunks(h)
    ngrp = (nch + GRP - 1) // GRP
    k = float(t / CASE)
    (p_ina, p_inb, p_ps, p_m, p_mt, p_out) = pools

    xoff = _X_OFF[tag]
    xr3 = xin[xoff:xoff + B_PER * 255 * hw] \
        .rearrange("(b c s) -> c b s", b=B_PER, c=255)

    def seg_dma(dst_tile, nrows, src0, c0, w):
        done = 0
        while done < w:
            cell = c0 + done
            b = cell // hw
            s = cell % hw
            span = min(w - done, hw - s)
            nc.sync.dma_start(dst_tile[0:nrows, done:done + span],
                              xr3[src0:src0 + nrows, b, s:s + span])
            done += span

    for g in range(ngrp):
        j0 = g * GRP
        gc = min(GRP, nch - j0)
        c0 = j0 * CHUNK
        w = min(GRP * CHUNK, n - c0)

        in_a = p_ina.tile([128, GRP * CHUNK], F32, tag="in_a")
        in_b = p_inb.tile([128, GRP * CHUNK], F32, tag="in_b")
        seg_dma(in_a, 128, 0, c0, w)
        seg_dma(in_b, 127, 128, c0, w)

        ps = p_ps.tile([128, 4 * 512], F32, tag="ps")
        pg = ps[:].rearrange("p (g x) -> p g x", g=4)[:, 0:gc, :]
        ncs = []
        for jj in range(gc):
            ncj = min(CHUNK, w - jj * CHUNK)
            ncs.append(ncj)
            if ncj < CHUNK:
                nc.vector.memset(ps[:, jj * 512:jj * 512 + 255], 0.0)
            nc.tensor.transpose(ps[0:ncj, jj * 512:jj * 512 + 128],
                                in_a[:, jj * CHUNK:jj * CHUNK + ncj],
                                sb["iden"])
            nc.tensor.matmul(ps[0:ncj, jj * 512 + 128:jj * 512 + 255],
                             in_b[0:127, jj * CHUNK:jj * CHUNK + ncj],
                             sb["iden"][0:127, 0:127],
                             is_transpose=True, start=False, stop=True,
                             skip_group_check=True)

        cls_ap = _a85(pg, 5, 80)          # [128, gc, 3, 80]
        conf_ap = _a85(pg, 0).squeeze(3)  # [128, gc, 3]

        # ---- scan 1: exact class max ----
        m_sb = p_m.tile([128, 12], F32, tag="m_sb")
        m_v = m_sb[:].rearrange("p (g a) -> p g a", g=4)[:, 0:gc, :]
        if ST >= 2:
            nc.vector.tensor_reduce(m_v, cls_ap, axis=AX.X, op=OP.max)
        else:
            nc.vector.memset(m_sb[:, :], 0.0)

        # ---- exact 3-term bf16 split of m (gpsimd, off critical engines):
        # m = h1 + h2 + h3 with every term bf16-representable.
        BF16 = mybir.dt.bfloat16
        hb = p_m.tile([128, 12], BF16, tag="hb")
        hb2 = p_m.tile([128, 12], BF16, tag="hb2")
        r1 = p_m.tile([128, 12], F32, tag="r1")
        msp = p_m.tile([128, 128], F32, tag="msp")
        hb_v = hb[:].rearrange("p (g a) -> p g a", g=4)[:, 0:gc, :]
        hb2_v = hb2[:].rearrange("p (g a) -> p g a", g=4)[:, 0:gc, :]
        r1_v = r1[:].rearrange("p (g a) -> p g a", g=4)[:, 0:gc, :]
        mspv = msp[:].rearrange("p (g r) -> p g r", g=4)
        if ST >= 3:
            nc.vector.memset(msp[:, :], 0.0)
            nc.vector.tensor_copy(hb_v, m_v)
            nc.vector.tensor_copy(mspv[:, 0:gc, 0:3], hb_v)
            nc.vector.tensor_tensor(r1_v, m_v, hb_v, op=OP.subtract)
            nc.vector.tensor_copy(hb2_v, r1_v)
            nc.vector.tensor_copy(mspv[:, 0:gc, 3:6], hb2_v)
            nc.vector.tensor_tensor(mspv[:, 0:gc, 6:9], r1_v, hb2_v,
                                    op=OP.subtract)

        # ---- transpose m-split into psum spare (halves: bases 0/32) ----
        mts = []
        for hh in range((gc + 1) // 2 if ST >= 4 else 0):
            nc.tensor.matmul(ps[0:64, hh * 512 + 256:hh * 512 + 384],
                             msp[:, 64 * hh:64 * hh + 64],
                             sb["iden"][0:128, 0:128],
                             is_transpose=True, start=False, stop=True,
                             skip_group_check=True)
            mt_t = p_mt.tile([64, 128], BF16, tag=f"mtsb{hh}")
            nc.scalar.copy(mt_t[:, :],
                           ps[0:64, hh * 512 + 256:hh * 512 + 384])
            mts.append(mt_t)

        # ---- recenter: cls += -m, then += iota (separate accumulates) --
        for jj in range(gc if ST >= 5 else 0):
            out_cls = ps[:, jj * 512:jj * 512 + 255]
            bp = 32 * (jj % 2)
            nc.tensor.matmul(out_cls, mts[jj // 2][bp:bp + 9, :],
                             sb["sel9"][bp:bp + 9, 0:255],
                             start=False, stop=True, skip_group_check=True)
            nc.tensor.matmul(out_cls, sb["onesb"], sb["iotam"][:, 0:255],
                             start=False, stop=True, skip_group_check=True)

        # ---- scan 2: argmax ----
        idx_sb = p_m.tile([128, 12], F32, tag="idx_sb")
        idx_v = idx_sb[:].rearrange("p (g a) -> p g a", g=4)[:, 0:gc, :]
        if ST >= 6:
            nc.vector.tensor_reduce(idx_v, cls_ap, axis=AX.X, op=OP.max)
        else:
            nc.vector.memset(idx_sb[:, :], 0.0)

        # ---- decode ----
        out4 = p_out.tile([128, GRP * 18], F32, tag="out4")
        if ST < 7:
            nc.vector.memset(out4[:, :], 0.0)
        o4 = out4[:].rearrange("p (g a s) -> p g a s", g=4, a=3)
        o4t = out4[:].rearrange("p (g a s) -> p g s a", g=4, a=3)

        if ST >= 7:
            # conf = 1/(1 + exp(-logit)): stay in the Exp table set
            econf = p_m.tile([128, 12], F32, tag="econf")
            e_v = econf[:].rearrange("p (g a) -> p g a", g=4)[:, 0:gc, :]
            nc.scalar.activation(e_v, conf_ap, AF.Exp, scale=-1.0)
            ep1 = p_m.tile([128, 12], F32, tag="ep1")
            e1_v = ep1[:].rearrange("p (g a) -> p g a", g=4)[:, 0:gc, :]
            nc.vector.tensor_scalar(e1_v, e_v, 1.0, None, op0=OP.add)
            nc.vector.reciprocal(o4t[:, 0:gc, 0:1, :].squeeze(2), e1_v)

            gxy_ap = sb["gxy"][:, gxy_off + 2 * j0:gxy_off + 2 * j0 + 2 * gc]
            gxy_r = gxy_ap.rearrange("p (g q) -> p g q", q=2)
            for kk in range(2):
                g_v = gxy_r[:, :, kk:kk + 1].broadcast_to([128, gc, 3])
                src = _a85(pg, 1 + kk).squeeze(3)
                dst = o4t[:, 0:gc, 1 + kk:2 + kk, :].squeeze(2)
                nc.vector.scalar_tensor_tensor(dst, src, k, g_v,
                                               op0=OP.mult, op1=OP.add)

            twh = p_m.tile([128, 24], F32, tag="twh")
            twh_v = twh[:].rearrange("p (g q a) -> p g q a", g=4, q=2)
            for kk in range(2):
                nc.scalar.activation(
                    twh_v[:, 0:gc, kk:kk + 1, :].squeeze(2),
                    _a85(pg, 3 + kk).squeeze(3), AF.Exp)
            anch_v = sb["anch"].rearrange("p (q a) -> p q a", q=2) \
                .unsqueeze(1).broadcast_to([128, gc, 2, 3])
            nc.vector.tensor_tensor(o4t[:, 0:gc, 3:5, :],
                                    twh_v[:, 0:gc], anch_v, op=OP.mult)

            nc.scalar.activation(o4t[:, 0:gc, 5:6, :].squeeze(2), idx_v,
                                 AF.Copy, bias=79.0, scale=-(2.0 ** 31))

            for a in range(3):
                cb = conf_ap[:, :, a:a + 1].broadcast_to([128, gc, 6])
                dst = o4[:, 0:gc, a, :]
                nc.vector.scalar_tensor_tensor(dst, cb, 0.0, dst,
                                               op0=OP.is_gt, op1=OP.mult)

        nfull = sum(1 for x in ncs if x == CHUNK)
        r0 = _O_OFF[tag] + c0
        if nfull:
            dst = oX[r0:r0 + nfull * CHUNK, :] \
                .rearrange("(g p) c -> p g c", p=CHUNK)
            nc.sync.dma_start(dst, o4[:, 0:nfull].rearrange(
                "p g a s -> p g (a s)"))
        if nfull < gc:
            ncj = ncs[nfull]
            rp = r0 + nfull * CHUNK
            nc.sync.dma_start(oX[rp:rp + ncj, :],
                              out4[0:ncj, 18 * nfull:18 * nfull + 18])


def build():
    nc = bacc.Bacc("TRN2", target_bir_lowering=False, debug=False,
                   num_devices=N_CORES)
    xin = nc.dram_tensor("xin", [TOTAL_IN], F32, kind="ExternalInput").ap()
    oX = nc.dram_tensor("out", [TOTAL_OUT_ROWS, 18], F32,
                        kind="ExternalOutput").ap()

    with tile.TileContext(nc) as tc:
        with ExitStack() as ctx:
            p_c = ctx.enter_context(tc.tile_pool(name="consts", bufs=1))
            p_ina = ctx.enter_context(tc.tile_pool(name="inpa", bufs=4))
            p_inb = ctx.enter_context(tc.tile_pool(name="inpb", bufs=4))
            p_ps = ctx.enter_context(
                tc.tile_pool(name="ps", bufs=2, space="PSUM"))
            p_m = ctx.enter_context(tc.tile_pool(name="small", bufs=3))
            p_mt = ctx.enter_context(tc.tile_pool(name="mt", bufs=3))
            p_out = ctx.enter_context(tc.tile_pool(name="out", bufs=4))

            shapes = {"gxy": [128, _CONSTS["gxy"].shape[1]],
                      "iden": [128, 128], "sel9": [128, 128],
                      "iotam": [1, 128], "onesb": [1, 64],
                      "anch": [128, 18]}
            sb = {}
            for name, shp in shapes.items():
                t_ = p_c.tile(shp, F32, tag=name)
                size = shp[0] * shp[1]
                src = xin[_CONST_OFF[name]:_CONST_OFF[name] + size] \
                    .rearrange("(p f) -> p f", p=shp[0])
                nc.sync.dma_start(t_[:], src)
                if name in ("sel9", "iotam", "onesb"):
                    sb[name] = t_[:].bitcast(mybir.dt.bfloat16)
                else:
                    sb[name] = t_[:]
            anch_t = sb["anch"]

            pools = (p_ina, p_inb, p_ps, p_m, p_mt, p_out)
            for _rep in range(int(os.environ.get("KREP", "1"))):
                gxy_off = 0
                anch_off = 0
                for tag, h, t in SCALES:
                    sbs = dict(sb)
                    sbs["anch"] = anch_t[:, anch_off:anch_off + 6]
                    _emit_scale(nc, tc, ctx, pools, sbs, xin, oX, h, t,
                                tag, gxy_off)
                    gxy_off += 2 * _nchunks(h)
                    anch_off += 6
    nc.compile()
    return nc


_NC = None


def _get_nc():
    global _NC
    if _NC is None:
        _NC = build()
    return _NC


def _make_anch(anchors):
    anch = np.zeros((128, 18), np.float32)
    off = 0
    for tag, h, _ in SCALES:
        a = anchors[tag].astype(np.float64) / CASE
        for kk in range(2):
            for aa in range(3):
                anch[:, off + kk * 3 + aa] = a[aa, kk]
        off += 6
    return anch


def _pack_core(xs, anch):
    parts = [np.asarray(xs["52"]).ravel(), np.asarray(xs["26"]).ravel(),
             np.asarray(xs["13"]).ravel(),
             _CONSTS["gxy"].ravel(), _CONSTS["iden"].ravel(),
             _CONSTS["sel9"].ravel(), _CONSTS["iotam"].ravel(),
             _CONSTS["onesb"].ravel(), anch.ravel()]
    out = np.concatenate(parts)
    assert out.size == TOTAL_IN and out.dtype == np.float32
    return out


def kernel(out13, out26, out52, anchors13, anchors26, anchors52):
    nc = _get_nc()
    xs_all = {"13": np.asarray(out13), "26": np.asarray(out26),
              "52": np.asarray(out52)}
    anchors = {"13": np.asarray(anchors13), "26": np.asarray(anchors26),
               "52": np.asarray(anchors52)}
    anch = _make_anch(anchors)

    in_maps = []
    for i in range(N_CORES):
        xs = {tag: xs_all[tag][i * B_PER:(i + 1) * B_PER]
              for tag, _, _ in SCALES}
        in_maps.append({"xin": _pack_core(xs, anch)})

    res = run_bass_kernel_spmd(nc, in_maps, list(range(N_CORES))).results

    parts = []
    for tag, h, _ in SCALES[::-1]:  # output order: 13, 26, 52
        o0 = _O_OFF[tag]
        for i in range(N_CORES):
            parts.append(res[i]["out"][o0:o0 + _cells(h)].reshape(-1, 6))
    return np.concatenate(parts, axis=0)



# revision 2
# speedup vs baseline: 92.8845x; 92.8845x over previous
"""YOLO-style detection decode (nms_detection) on 8 trn2 NeuronCores.

Data-parallel over batch (64 -> 8 images/core). Per-core inputs are
packed into ONE flat f32 DRAM tensor, with each scale's activations
pre-transposed on the host to channel-major [255, cells] so every
device load is a single large contiguous-stride DMA (~1-4 MB). The
result is ONE [128, 4014] f32 tensor (cells chunk-major on partitions,
18 = 3 anchors x 6 box floats per cell), reassembled on the host.

Device pipeline per 4-chunk group (chunk = 128 cells):
  - PE transposes chunks from the big SBUF strips into PSUM
    -> [cell, 255ch].
  - DVE reduce_max over the 80 class cols per anchor -> m (exact).
  - m is split into 3 exact bf16 terms (ACT casts + DVE subtracts),
    PE-transposed, and a K=9 bf16 matmul subtracts m from the class
    logits (exact: Sterbenz near the max); a K=1 matmul adds
    (79-c)*2^-31. The winner's value is then exactly
    (79-argmax)*2^-31 >= 0 while every loser stays < 0, so a second
    DVE reduce_max recovers argmax exactly (incl. first-index ties,
    matching jnp.argmax).
  - The 5 box channels (conf,x,y,w,h per anchor) are copied out of
    PSUM by the scalar engine into per-scale SBUF accumulators; both
    scans also write per-scale accumulators.
  - Decode (sigmoid, grid offsets, exp*anchor, conf mask) runs ONCE
    per scale as ~10 wide DVE/ACT ops over the accumulators, writing
    the output accumulator in final layout; one DMA stores it.
"""

import os
from contextlib import ExitStack

import numpy as np

import concourse.bass as bass
import concourse.tile as tile
from concourse import bacc, mybir
from concourse.bass_utils import run_bass_kernel_spmd

N_CORES = 8
B = 64
B_PER = B // N_CORES
CASE = 416.0
SCALES = [("52", 52, 8.0), ("26", 26, 16.0), ("13", 13, 32.0)]
CHUNK = 128
GRP = 4            # chunks per PSUM group
LDC = 32           # chunks per SBUF load strip (4096 cells)
F32 = mybir.dt.float32
BF16 = mybir.dt.bfloat16
AX = mybir.AxisListType
OP = mybir.AluOpType
AF = mybir.ActivationFunctionType
IOTA_SCALE = 2.0 ** -31


def _cells(h):
    return B_PER * h * h


def _nchunks(h):
    return (_cells(h) + CHUNK - 1) // CHUNK


NCH = {tag: _nchunks(h) for tag, h, _ in SCALES}
NCH_TOT = sum(NCH.values())          # 223
OUT_W = NCH_TOT * 18                 # 4014 f32 per partition


def _gxy_section(h, t):
    """[128, 2*nch] grid offsets: cols (2j, 2j+1) = (gx, gy) of chunk j."""
    n = _cells(h)
    nch = _nchunks(h)
    cells = np.arange(nch * CHUNK)
    s = cells % (h * h)
    gx = (s % h).astype(np.float64) * t / CASE
    gy = (s // h).astype(np.float64) * t / CASE
    gx[cells >= n] = 0.0
    gy[cells >= n] = 0.0
    out = np.zeros((CHUNK, 2 * nch), np.float32)
    for j in range(nch):
        out[:, 2 * j] = gx[j * CHUNK:(j + 1) * CHUNK]
        out[:, 2 * j + 1] = gy[j * CHUNK:(j + 1) * CHUNK]
    return out


def _consts():
    import ml_dtypes
    bf = ml_dtypes.bfloat16
    # transposed chunk layout: anchor a's class cols at 85a+5 .. 85a+85.
    # sel9 row (3*term + a): -1 selector for the 3-term bf16 split of m.
    sel9 = np.zeros((9, 256), bf)
    for term in range(3):
        for a in range(3):
            sel9[3 * term + a, 85 * a + 5:85 * a + 85] = -1.0
    iotam = np.zeros((1, 256), bf)
    for a in range(3):
        iotam[0, 85 * a + 5:85 * a + 85] = \
            ((79.0 - np.arange(80)) * IOTA_SCALE).astype(bf)
    onesb = np.ones((1, 128), bf)
    iden = np.eye(128, dtype=np.float32)
    gxy = np.concatenate([_gxy_section(h, t) for _, h, t in SCALES], axis=1)
    return {
        "gxy": gxy.astype(np.float32),
        "iden": iden,
        "sel9": sel9.view(np.float32),
        "iotam": iotam.view(np.float32),
        "onesb": onesb.view(np.float32),
    }


_CONSTS = _consts()
_CONST_SHAPES = {"gxy": [128, 2 * NCH_TOT], "iden": [128, 128],
                 "sel9": [9, 128], "iotam": [1, 128], "onesb": [1, 64],
                 "anch": [128, 18]}

# packed input layout (f32 elements, per core): per-scale channel-major
# activations, then the small constants.
_X_OFF = {}
_off = 0
for _tag, _h, _t in SCALES:
    _X_OFF[_tag] = _off
    _off += 255 * _cells(_h)
_CONST_OFF = {}
for _name in ("gxy", "iden", "sel9", "iotam", "onesb", "anch"):
    _CONST_OFF[_name] = _off
    _off += int(np.prod(_CONST_SHAPES[_name]))
TOTAL_IN = _off

# chunk-column base per scale in the accumulators / output
_J_OFF = {}
_off = 0
for _tag, _h, _t in SCALES:
    _J_OFF[_tag] = _off
    _off += NCH[_tag]


def _emit_scale(nc, tc, sb, acc, xin, h, t, tag):
    n = _cells(h)
    nch = NCH[tag]
    J0 = _J_OFF[tag]
    k = float(t / CASE)
    p_in, p_ps, p_m, p_dec = acc["pools"]
    boxacc, idxacc, outacc = acc["boxacc"], acc["idxacc"], acc["outacc"]

    xc = xin[_X_OFF[tag]:_X_OFF[tag] + 255 * n] \
        .rearrange("(c s) -> c s", c=255)

    nld = (nch + LDC - 1) // LDC
    for ld in range(nld):
        jb = ld * LDC                      # first chunk of this strip
        lc = min(LDC, nch - jb)            # chunks in strip
        c0 = jb * CHUNK
        w = min(lc * CHUNK, n - c0)        # valid cells in strip

        in_a = p_in.tile([128, LDC * CHUNK], F32, tag="in_a")
        in_b = p_in.tile([128, LDC * CHUNK], F32, tag="in_b")
        nc.sync.dma_start(in_a[0:128, 0:w], xc[0:128, c0:c0 + w])
        nc.sync.dma_start(in_b[0:127, 0:w], xc[128:255, c0:c0 + w])

        for g in range((lc + GRP - 1) // GRP):
            jl = g * GRP                   # first chunk (strip-local)
            gc = min(GRP, lc - jl)
            J = J0 + jb + jl               # first chunk (global col)

            ps = p_ps.tile([128, GRP * 512], F32, tag="ps")
            pg = ps[:].rearrange("p (g x) -> p g x", g=GRP)[:, 0:gc, :]
            for jj in range(gc):
                cs = (jl + jj) * CHUNK
                ncj = min(CHUNK, w - cs)
                if ncj < CHUNK:
                    nc.vector.memset(ps[:, jj * 512:jj * 512 + 255], 0.0)
                nc.tensor.transpose(ps[0:ncj, jj * 512:jj * 512 + 128],
                                    in_a[:, cs:cs + ncj], sb["iden"])
                nc.tensor.matmul(ps[0:ncj, jj * 512 + 128:jj * 512 + 255],
                                 in_b[0:127, cs:cs + ncj],
                                 sb["iden"][0:127, 0:127],
                                 is_transpose=True, start=False, stop=True,
                                 skip_group_check=True)

            cham = pg[:, :, 0:255].rearrange("p g (a r) -> p g a r",
                                             a=3, r=85)
            cls_ap = cham[:, :, :, 5:85]          # [128, gc, 3, 80]

            # box channels (conf,x,y,w,h per anchor) -> accumulator
            nc.scalar.copy(boxacc[:, J:J + gc], cham[:, :, :, 0:5])

            # ---- scan 1: exact per-anchor class max ----
            m_sb = p_m.tile([128, GRP * 3], F32, tag="m_sb")
            m_v = m_sb[:].rearrange("p (g a) -> p g a", g=GRP)[:, 0:gc, :]
            nc.vector.tensor_reduce(m_v, cls_ap, axis=AX.X, op=OP.max)

            # ---- exact 3-term bf16 split: m = h1 + h2 + r2 ----
            hb = p_m.tile([128, GRP * 3], BF16, tag="hb")
            hb2 = p_m.tile([128, GRP * 3], BF16, tag="hb2")
            r1 = p_m.tile([128, GRP * 3], F32, tag="r1")
            msp = p_m.tile([128, GRP * 9], F32, tag="msp")
            hb_v = hb[:].rearrange("p (g a) -> p g a", g=GRP)[:, 0:gc, :]
            hb2_v = hb2[:].rearrange("p (g a) -> p g a", g=GRP)[:, 0:gc, :]
            r1_v = r1[:].rearrange("p (g a) -> p g a", g=GRP)[:, 0:gc, :]
            mspv = msp[:].rearrange("p (g r) -> p g r", g=GRP)[:, 0:gc, :]
            nc.scalar.copy(hb_v, m_v)
            nc.scalar.copy(mspv[:, :, 0:3], hb_v)
            nc.vector.tensor_tensor(r1_v, m_v, hb_v, op=OP.subtract)
            nc.scalar.copy(hb2_v, r1_v)
            nc.scalar.copy(mspv[:, :, 3:6], hb2_v)
            nc.vector.tensor_tensor(mspv[:, :, 6:9], r1_v, hb2_v,
                                    op=OP.subtract)

            # ---- transpose the split into psum spare cols ----
            for jj in range(gc):
                nc.tensor.matmul(
                    ps[0:9, jj * 512 + 256:jj * 512 + 384],
                    msp[:, jj * 9:jj * 9 + 9], sb["iden"],
                    is_transpose=True, start=False, stop=True,
                    skip_group_check=True)
            mt = p_m.tile([9, GRP * 128], BF16, tag="mt")
            mt_v = mt[0:9, 0:gc * 128].rearrange("p (g x) -> p g x", g=gc)
            nc.scalar.copy(
                mt_v, ps[0:9, :].rearrange("p (g x) -> p g x", g=GRP)
                [:, 0:gc, 256:384])

            # ---- recenter: cls += -m, then += iota payload ----
            for jj in range(gc):
                out_cls = ps[:, jj * 512:jj * 512 + 255]
                nc.tensor.matmul(out_cls, mt[0:9, jj * 128:jj * 128 + 128],
                                 sb["sel9"][0:9, 0:255],
                                 start=False, stop=True,
                                 skip_group_check=True)
                nc.tensor.matmul(out_cls, sb["onesb"], sb["iotam"][:, 0:255],
                                 start=False, stop=True,
                                 skip_group_check=True)

            # ---- scan 2: argmax payload ----
            nc.vector.tensor_reduce(idxacc[:, J:J + gc], cls_ap,
                                    axis=AX.X, op=OP.max)

    # ---- batched decode over the whole scale ----
    oA = outacc[:].rearrange("p (c a s) -> p c a s", a=3, s=6)[:, J0:J0 + nch]
    oT = outacc[:].rearrange("p (c a s) -> p c s a", a=3, s=6)[:, J0:J0 + nch]
    bx = boxacc[:, J0:J0 + nch]                   # [128, nch, 3, 5]

    econf = p_dec.tile([128, nch * 3], F32, tag=f"econf{tag}")
    e_v = econf[:].rearrange("p (c a) -> p c a", c=nch)
    nc.scalar.activation(e_v, bx[:, :, :, 0], AF.Exp, scale=-1.0)
    nc.vector.tensor_scalar(e_v, e_v, 1.0, None, op0=OP.add)
    nc.vector.reciprocal(oT[:, :, 0, :], e_v)

    gxy_r = sb["gxy"][:, 2 * J0:2 * (J0 + nch)] \
        .rearrange("p (c q) -> p c q", q=2)
    for kk in range(2):
        g_v = gxy_r[:, :, kk:kk + 1].broadcast_to([128, nch, 3])
        nc.vector.scalar_tensor_tensor(oT[:, :, 1 + kk, :],
                                       bx[:, :, :, 1 + kk], k, g_v,
                                       op0=OP.mult, op1=OP.add)

    twh = p_dec.tile([128, nch * 6], F32, tag=f"twh{tag}")
    twh_v = twh[:].rearrange("p (c q a) -> p c q a", c=nch, q=2)
    for kk in range(2):
        nc.scalar.activation(twh_v[:, :, kk, :], bx[:, :, :, 3 + kk], AF.Exp)
    anch_v = sb["anch"].rearrange("p (q a) -> p q a", q=2) \
        .unsqueeze(1).broadcast_to([128, nch, 2, 3])
    nc.vector.tensor_tensor(oT[:, :, 3:5, :], twh_v, anch_v, op=OP.mult)

    nc.scalar.activation(oT[:, :, 5, :], idxacc[:, J0:J0 + nch],
                         AF.Copy, bias=79.0, scale=-(2.0 ** 31))

    for a in range(3):
        cb = bx[:, :, a, 0:1].broadcast_to([128, nch, 6])
        dst = oA[:, :, a, :]
        nc.vector.scalar_tensor_tensor(dst, cb, 0.0, dst,
                                       op0=OP.is_gt, op1=OP.mult)


def build():
    nc = bacc.Bacc("TRN2", target_bir_lowering=False, debug=False,
                   num_devices=N_CORES)
    xin = nc.dram_tensor("xin", [TOTAL_IN], F32, kind="ExternalInput").ap()
    oX = nc.dram_tensor("out", [128, OUT_W], F32,
                        kind="ExternalOutput").ap()

    with tile.TileContext(nc) as tc:
        with ExitStack() as ctx:
            p_c = ctx.enter_context(tc.tile_pool(name="consts", bufs=1))
            p_in = ctx.enter_context(tc.tile_pool(name="inp", bufs=2))
            p_ps = ctx.enter_context(
                tc.tile_pool(name="ps", bufs=2, space="PSUM"))
            p_m = ctx.enter_context(tc.tile_pool(name="small", bufs=3))
            p_dec = ctx.enter_context(tc.tile_pool(name="dec", bufs=1))
            p_acc = ctx.enter_context(tc.tile_pool(name="acc", bufs=1))

            sb = {}
            for name, shp in _CONST_SHAPES.items():
                t_ = p_c.tile(shp, F32, tag=name)
                size = shp[0] * shp[1]
                src = xin[_CONST_OFF[name]:_CONST_OFF[name] + size] \
                    .rearrange("(p f) -> p f", p=shp[0])
                nc.sync.dma_start(t_[:], src)
                if name in ("sel9", "iotam", "onesb"):
                    sb[name] = t_[:].bitcast(mybir.dt.bfloat16)
                else:
                    sb[name] = t_[:]
            anch_t = sb["anch"]

            boxacc = p_acc.tile([128, NCH_TOT * 15], F32, tag="boxacc")
            boxv = boxacc[:].rearrange("p (c a s) -> p c a s", a=3, s=5)
            idxacc = p_acc.tile([128, NCH_TOT * 3], F32, tag="idxacc")
            idxv = idxacc[:].rearrange("p (c a) -> p c a", a=3)
            outacc = p_acc.tile([128, OUT_W], F32, tag="outacc")
            acc = {"pools": (p_in, p_ps, p_m, p_dec),
                   "boxacc": boxv, "idxacc": idxv, "outacc": outacc}

            for _rep in range(int(os.environ.get("KREP", "1"))):
                anch_off = 0
                for tag, h, t in SCALES:
                    sbs = dict(sb)
                    sbs["anch"] = anch_t[:, anch_off:anch_off + 6]
                    _emit_scale(nc, tc, sbs, acc, xin, h, t, tag)
                    anch_off += 6
                nc.sync.dma_start(oX, outacc[:])
    nc.compile()
    return nc


_NC = None


def _get_nc():
    global _NC
    if _NC is None:
        _NC = build()
    return _NC


def _make_anch(anchors):
    anch = np.zeros((128, 18), np.float32)
    off = 0
    for tag, h, _ in SCALES:
        a = anchors[tag].astype(np.float64) / CASE
        for kk in range(2):
            for aa in range(3):
                anch[:, off + kk * 3 + aa] = a[aa, kk]
        off += 6
    return anch


def _pack_core(xs, anch):
    parts = []
    for tag, h, _ in SCALES:
        a = np.ascontiguousarray(
            np.asarray(xs[tag]).reshape(B_PER, 255, h * h).transpose(1, 0, 2))
        parts.append(a.ravel())
    parts += [_CONSTS["gxy"].ravel(), _CONSTS["iden"].ravel(),
              _CONSTS["sel9"].ravel(), _CONSTS["iotam"].ravel(),
              _CONSTS["onesb"].ravel(), anch.ravel()]
    out = np.concatenate(parts)
    assert out.size == TOTAL_IN and out.dtype == np.float32
    return out


def _unpack_core(res):
    """[128, 4014] device tensor -> [28392, 18] cell-major rows."""
    parts = []
    for tag, h, _ in SCALES[::-1]:  # output order: 13, 26, 52
        J0 = _J_OFF[tag]
        nch = NCH[tag]
        blk = res[:, 18 * J0:18 * (J0 + nch)].reshape(128, nch, 18)
        rows = blk.transpose(1, 0, 2).reshape(nch * CHUNK, 18)
        parts.append(rows[:_cells(h)])
    return parts


def kernel(out13, out26, out52, anchors13, anchors26, anchors52):
    nc = _get_nc()
    xs_all = {"13": np.asarray(out13), "26": np.asarray(out26),
              "52": np.asarray(out52)}
    anchors = {"13": np.asarray(anchors13), "26": np.asarray(anchors26),
               "52": np.asarray(anchors52)}
    anch = _make_anch(anchors)

    in_maps = []
    for i in range(N_CORES):
        xs = {tag: xs_all[tag][i * B_PER:(i + 1) * B_PER]
              for tag, _, _ in SCALES}
        in_maps.append({"xin": _pack_core(xs, anch)})

    res = run_bass_kernel_spmd(nc, in_maps, list(range(N_CORES))).results

    per_core = [_unpack_core(res[i]["out"]) for i in range(N_CORES)]
    parts = []
    for si in range(len(SCALES)):
        for i in range(N_CORES):
            parts.append(per_core[i][si].reshape(-1, 6))
    return np.concatenate(parts, axis=0)


# revision 6
# speedup vs baseline: 137.1867x; 1.4770x over previous
"""YOLO-style detection decode (nms_detection) on 8 trn2 NeuronCores.

Data-parallel over batch (64 -> 8 images/core). The host packs each
core's inputs into ONE flat f32 DRAM tensor, pre-transposed per scale
to cell-major chunks: [128 partitions, nch * 255] where partition p,
column 255*j + c holds channel c of cell j*128 + p. Every device load
is then one large contiguous-stride DMA, and no on-device transpose is
needed at all (the memory-regime roofline is the 29 MB input stream).
The result is ONE [128, 4014] f32 tensor (chunk-major cells on
partitions, 18 = 3 anchors x 6 box floats per cell), reassembled on
the host.

Device pipeline per 32-chunk group (chunk = 128 cells), all reads
straight from the input strip in SBUF:
  - scalar engine copies the 5 box channels (conf,x,y,w,h per anchor)
    into a per-scale accumulator.
  - DVE reduce_max over the 80 class cols per anchor -> m (exact).
  - DVE computes cls - m into an SBUF scratch (exact at the top:
    x - x = 0, Sterbenz near the max), then adds (79-c)*2^-31. The
    winner's value is exactly (79-argmax)*2^-31 >= 0 while every loser
    stays < 0, so a second DVE reduce_max recovers argmax exactly
    (incl. first-index ties, matching jnp.argmax).
  - Decode (sigmoid, grid offsets, exp*anchor, conf mask) runs ONCE
    per scale as ~10 wide DVE/ACT ops over the accumulators, writing
    the output accumulator in final layout; one DMA stores it.
"""

import os
from contextlib import ExitStack

import numpy as np

import concourse.bass as bass
import concourse.tile as tile
from concourse import bacc, mybir
from concourse.bass_utils import run_bass_kernel_spmd

N_CORES = 8
B = 64
B_PER = B // N_CORES
CASE = 416.0
SCALES = [("52", 52, 8.0), ("26", 26, 16.0), ("13", 13, 32.0)]
CHUNK = 128
LDC = 32           # chunks per SBUF load strip / compute group
F32 = mybir.dt.float32
AX = mybir.AxisListType
OP = mybir.AluOpType
AF = mybir.ActivationFunctionType
IOTA_SCALE = 2.0 ** -31


def _cells(h):
    return B_PER * h * h


def _nchunks(h):
    return (_cells(h) + CHUNK - 1) // CHUNK


NCH = {tag: _nchunks(h) for tag, h, _ in SCALES}
NCH_TOT = sum(NCH.values())          # 223
OUT_W = NCH_TOT * 18                 # 4014 f32 per partition


def _gxy_section(h, t):
    """[128, 2*nch] grid offsets: cols (2j, 2j+1) = (gx, gy) of chunk j."""
    n = _cells(h)
    nch = _nchunks(h)
    cells = np.arange(nch * CHUNK)
    s = cells % (h * h)
    gx = (s % h).astype(np.float64) * t / CASE
    gy = (s // h).astype(np.float64) * t / CASE
    gx[cells >= n] = 0.0
    gy[cells >= n] = 0.0
    out = np.zeros((CHUNK, 2 * nch), np.float32)
    for j in range(nch):
        out[:, 2 * j] = gx[j * CHUNK:(j + 1) * CHUNK]
        out[:, 2 * j + 1] = gy[j * CHUNK:(j + 1) * CHUNK]
    return out


def _consts():
    iota = np.broadcast_to(
        ((79.0 - np.arange(80)) * IOTA_SCALE).astype(np.float32), (128, 80))
    gxy = np.concatenate([_gxy_section(h, t) for _, h, t in SCALES], axis=1)
    return {"gxy": gxy.astype(np.float32),
            "iota": np.ascontiguousarray(iota)}


_CONSTS = _consts()
_CONST_SHAPES = {"gxy": [128, 2 * NCH_TOT], "iota": [128, 80],
                 "anch": [128, 18]}

# packed input layout (f32 elements, per core): per-scale cell-major
# chunked activations, then the small constants.
_X_OFF = {}
_off = 0
for _tag, _h, _t in SCALES:
    _X_OFF[_tag] = _off
    _off += NCH[_tag] * 255 * CHUNK
_CONST_OFF = {}
for _name in ("gxy", "iota", "anch"):
    _CONST_OFF[_name] = _off
    _off += int(np.prod(_CONST_SHAPES[_name]))
TOTAL_IN = _off

# chunk-column base per scale in the accumulators / output
_J_OFF = {}
_off = 0
for _tag, _h, _t in SCALES:
    _J_OFF[_tag] = _off
    _off += NCH[_tag]


def _emit_scale(nc, tc, sb, acc, xin, h, t, tag):
    ST = int(os.environ.get("KSTAGE", "6"))
    TT = getattr(nc, os.environ.get("KTTENG", "vector"))
    nch = NCH[tag]
    J0 = _J_OFF[tag]
    k = float(t / CASE)
    p_in, p_cls, p_m, p_dec = acc["pools"]
    boxacc, idxacc, outacc = acc["boxacc"], acc["idxacc"], acc["outacc"]

    # [128, nch, 255]: partition p, chunk j, channel c = cell j*128+p
    xc = xin[_X_OFF[tag]:_X_OFF[tag] + nch * 255 * CHUNK] \
        .rearrange("(p j c) -> p j c", p=128, c=255)

    for jb in range(0, nch, LDC):
        lc = min(LDC, nch - jb)

        in_a = p_in.tile([128, LDC * 255], F32, tag="in_a")
        ia = in_a[:].rearrange("p (j c) -> p j c", c=255)[:, 0:lc, :]
        nc.sync.dma_start(ia, xc[:, jb:jb + lc, :])
        if ST < 1:
            continue

        iv = ia.rearrange("p j (a r) -> p j a r", a=3)   # [128, lc, 3, 85]
        cls_in = iv[:, :, :, 5:85]
        J = J0 + jb

        # box channels (conf,x,y,w,h per anchor) -> accumulator
        m_sb = p_m.tile([128, LDC * 3], F32, tag="m_sb")
        m_v = m_sb[:].rearrange("p (j a) -> p j a", j=LDC)[:, 0:lc, :]
        if ST >= 2:
            nc.scalar.copy(boxacc[:, J:J + lc], iv[:, :, :, 0:5])
            # ---- scan 1: exact per-anchor class max ----
            nc.vector.tensor_reduce(m_v, cls_in, axis=AX.X, op=OP.max)
        else:
            nc.vector.memset(m_sb[:, :], 0.0)

        # ---- recenter into scratch: cls - m, then + iota payload ----
        cls_s = p_cls.tile([128, LDC * 240], F32, tag="cls_s")
        cv = cls_s[:].rearrange("p (j a r) -> p j a r", j=LDC, a=3)[:, 0:lc]
        if ST >= 3:
            m_b = m_v.unsqueeze(3).broadcast_to([128, lc, 3, 80])
            TT.scalar_tensor_tensor(cv, cls_in, 1.0, m_b,
                                    op0=OP.mult, op1=OP.subtract)
        else:
            nc.vector.memset(cls_s[:, :], 0.0)
        if ST >= 4:
            i_b = sb["iota"].unsqueeze(1).unsqueeze(1) \
                .broadcast_to([128, lc, 3, 80])
            TT.tensor_tensor(cv, cv, i_b, op=OP.add)

        # ---- scan 2: argmax payload ----
        if ST >= 5:
            nc.vector.tensor_reduce(idxacc[:, J:J + lc], cv,
                                    axis=AX.X, op=OP.max)

    # ---- batched decode over the whole scale ----
    if ST < 6:
        return
    oA = outacc[:].rearrange("p (c a s) -> p c a s", a=3, s=6)[:, J0:J0 + nch]
    oT = outacc[:].rearrange("p (c a s) -> p c s a", a=3, s=6)[:, J0:J0 + nch]
    bx = boxacc[:, J0:J0 + nch]                   # [128, nch, 3, 5]

    econf = p_dec.tile([128, nch * 3], F32, tag=f"econf{tag}")
    e_v = econf[:].rearrange("p (c a) -> p c a", c=nch)
    nc.scalar.activation(e_v, bx[:, :, :, 0], AF.Exp, scale=-1.0)
    nc.vector.tensor_scalar(e_v, e_v, 1.0, None, op0=OP.add)
    nc.vector.reciprocal(oT[:, :, 0, :], e_v)

    gxy_r = sb["gxy"][:, 2 * J0:2 * (J0 + nch)] \
        .rearrange("p (c q) -> p c q", q=2)
    for kk in range(2):
        g_v = gxy_r[:, :, kk:kk + 1].broadcast_to([128, nch, 3])
        nc.vector.scalar_tensor_tensor(oT[:, :, 1 + kk, :],
                                       bx[:, :, :, 1 + kk], k, g_v,
                                       op0=OP.mult, op1=OP.add)

    twh = p_dec.tile([128, nch * 6], F32, tag=f"twh{tag}")
    twh_v = twh[:].rearrange("p (c q a) -> p c q a", c=nch, q=2)
    for kk in range(2):
        nc.scalar.activation(twh_v[:, :, kk, :], bx[:, :, :, 3 + kk], AF.Exp)
    anch_v = sb["anch"].rearrange("p (q a) -> p q a", q=2) \
        .unsqueeze(1).broadcast_to([128, nch, 2, 3])
    nc.vector.tensor_tensor(oT[:, :, 3:5, :], twh_v, anch_v, op=OP.mult)

    nc.scalar.activation(oT[:, :, 5, :], idxacc[:, J0:J0 + nch],
                         AF.Copy, bias=79.0, scale=-(2.0 ** 31))

    for a in range(3):
        cb = bx[:, :, a, 0:1].broadcast_to([128, nch, 6])
        dst = oA[:, :, a, :]
        nc.vector.scalar_tensor_tensor(dst, cb, 0.0, dst,
                                       op0=OP.is_gt, op1=OP.mult)


def build():
    nc = bacc.Bacc("TRN2", target_bir_lowering=False, debug=False,
                   num_devices=N_CORES)
    xin = nc.dram_tensor("xin", [TOTAL_IN], F32, kind="ExternalInput").ap()
    oX = nc.dram_tensor("out", [128, OUT_W], F32,
                        kind="ExternalOutput").ap()

    with tile.TileContext(nc) as tc:
        with ExitStack() as ctx:
            p_c = ctx.enter_context(tc.tile_pool(name="consts", bufs=1))
            p_in = ctx.enter_context(tc.tile_pool(name="inp", bufs=2))
            p_cls = ctx.enter_context(tc.tile_pool(name="cls", bufs=2))
            p_m = ctx.enter_context(tc.tile_pool(name="small", bufs=2))
            p_dec = ctx.enter_context(tc.tile_pool(name="dec", bufs=1))
            p_acc = ctx.enter_context(tc.tile_pool(name="acc", bufs=1))

            sb = {}
            for name, shp in _CONST_SHAPES.items():
                t_ = p_c.tile(shp, F32, tag=name)
                size = shp[0] * shp[1]
                src = xin[_CONST_OFF[name]:_CONST_OFF[name] + size] \
                    .rearrange("(p f) -> p f", p=shp[0])
                nc.sync.dma_start(t_[:], src)
                sb[name] = t_[:]
            anch_t = sb["anch"]

            _st = int(os.environ.get("KSTAGE", "6"))
            boxacc = p_acc.tile([128, NCH_TOT * 15], F32, tag="boxacc")
            boxv = boxacc[:].rearrange("p (c a s) -> p c a s", a=3, s=5)
            idxacc = p_acc.tile([128, NCH_TOT * 3], F32, tag="idxacc")
            idxv = idxacc[:].rearrange("p (c a) -> p c a", a=3)
            outacc = p_acc.tile([128, OUT_W], F32, tag="outacc")
            if _st < 6:
                nc.vector.memset(outacc[:, :], 0.0)
                nc.vector.memset(boxacc[:, :], 0.0)
                nc.vector.memset(idxacc[:, :], 0.0)
            acc = {"pools": (p_in, p_cls, p_m, p_dec),
                   "boxacc": boxv, "idxacc": idxv, "outacc": outacc}

            for _rep in range(int(os.environ.get("KREP", "1"))):
                anch_off = 0
                for tag, h, t in SCALES:
                    sbs = dict(sb)
                    sbs["anch"] = anch_t[:, anch_off:anch_off + 6]
                    _emit_scale(nc, tc, sbs, acc, xin, h, t, tag)
                    anch_off += 6
                nc.sync.dma_start(oX, outacc[:])
    nc.compile()
    return nc


_NC = None


def _get_nc():
    global _NC
    if _NC is None:
        _NC = build()
    return _NC


def _make_anch(anchors):
    anch = np.zeros((128, 18), np.float32)
    off = 0
    for tag, h, _ in SCALES:
        a = anchors[tag].astype(np.float64) / CASE
        for kk in range(2):
            for aa in range(3):
                anch[:, off + kk * 3 + aa] = a[aa, kk]
        off += 6
    return anch


def _pack_core(xs, anch):
    parts = []
    for tag, h, _ in SCALES:
        n = _cells(h)
        nch = NCH[tag]
        a = np.asarray(xs[tag]).reshape(B_PER, 255, h * h) \
            .transpose(0, 2, 1).reshape(n, 255)
        if nch * CHUNK > n:
            a = np.concatenate(
                [a, np.zeros((nch * CHUNK - n, 255), np.float32)], axis=0)
        a = np.ascontiguousarray(
            a.reshape(nch, CHUNK, 255).transpose(1, 0, 2))
        parts.append(a.ravel())
    parts += [_CONSTS["gxy"].ravel(), _CONSTS["iota"].ravel(), anch.ravel()]
    out = np.concatenate(parts)
    assert out.size == TOTAL_IN and out.dtype == np.float32
    return out


def _unpack_core(res):
    """[128, 4014] device tensor -> per-scale [n, 18] cell-major rows."""
    parts = []
    for tag, h, _ in SCALES[::-1]:  # output order: 13, 26, 52
        J0 = _J_OFF[tag]
        nch = NCH[tag]
        blk = res[:, 18 * J0:18 * (J0 + nch)].reshape(128, nch, 18)
        rows = blk.transpose(1, 0, 2).reshape(nch * CHUNK, 18)
        parts.append(rows[:_cells(h)])
    return parts


def kernel(out13, out26, out52, anchors13, anchors26, anchors52):
    nc = _get_nc()
    xs_all = {"13": np.asarray(out13), "26": np.asarray(out26),
              "52": np.asarray(out52)}
    anchors = {"13": np.asarray(anchors13), "26": np.asarray(anchors26),
               "52": np.asarray(anchors52)}
    anch = _make_anch(anchors)

    in_maps = []
    for i in range(N_CORES):
        xs = {tag: xs_all[tag][i * B_PER:(i + 1) * B_PER]
              for tag, _, _ in SCALES}
        in_maps.append({"xin": _pack_core(xs, anch)})

    res = run_bass_kernel_spmd(nc, in_maps, list(range(N_CORES))).results

    per_core = [_unpack_core(res[i]["out"]) for i in range(N_CORES)]
    parts = []
    for si in range(len(SCALES)):
        for i in range(N_CORES):
            parts.append(per_core[i][si].reshape(-1, 6))
    return np.concatenate(parts, axis=0)
